# revision 1
# baseline (speedup 1.0000x reference)
"""GAT (2-layer, 4-head then 1-head) Bass kernel for TRN2, 8-way graph-parallel.

Strategy per core (cores own contiguous dst-node shards):
  - build1: h1 = x @ W1 plus dense per-node attention scores via augmented rhs
    [W1 | W1@att_src_blocks | W1@att_dst_blocks]; h1 -> bf16 row table in DRAM
    (256B rows, dma_gather-able), scores -> fp32 sc_tab.
  - aggregation: edges sorted by dst into 128-dst windows; per 128-edge chunk,
    dma_gather h[src] rows (edge-major [128, slot, 128]); per-edge scores
    a_d[dst] (+) a_s[src] via two indirect DMA gathers (second with CCE add);
    Lrelu+Exp on ACT (batched per group); one-hot matrices generated on DVE by
    iota==dstlocal compare; weighted messages via DVE mul; scatter-add to
    dst-windows via TensorE matmul (onehot^T @ msg) accumulated in PSUM;
    denominators via onehot^T @ exp. Window drain: reciprocal, scale, +b1,
    relu, PE-transpose -> out1^T shard.
  - AllGather out1^T shards (bf16) across 8 cores.
  - build2/aggregation2: same machinery, heads=1, 64 features.
Output: per-core dst shard [N_c, 64] fp32; host concatenates.
"""

import math
from contextlib import ExitStack

import numpy as np
import ml_dtypes

import concourse.bass as bass
import concourse.mybir as mybir
import concourse.tile as tile

P = 128
FP32 = mybir.dt.float32
BF16 = mybir.dt.bfloat16
I16 = mybir.dt.int16
I32 = mybir.dt.int32
AF = mybir.ActivationFunctionType
OP = mybir.AluOpType

NEG_SLOPE = 0.2
DISABLE = set()


# ----------------------------------------------------------------------------
# Host-side planning (pure index/structure work; no tensor-value compute)
# ----------------------------------------------------------------------------

class Plan:
    pass


def make_plan(edge_index: np.ndarray, N: int, n_cores: int, group_windows: int = 2):
    """Shard edges by dst across cores, sort into 128-dst windows, pad each
    (window, table-half) to a chunk count shared by all cores, and build the
    per-core index/metadata arrays."""
    p = Plan()
    assert N % n_cores == 0
    Nc = N // n_cores
    n_win = math.ceil(Nc / P)
    # split table rows on a 128 boundary so build blocks don't straddle
    split = (N // 2 + P - 1) // P * P
    assert split < 32768 and (N - split) < 32768

    src = np.concatenate([edge_index[0], np.arange(N, dtype=np.int64)])
    dst = np.concatenate([edge_index[1], np.arange(N, dtype=np.int64)])
    src = src.astype(np.int64)
    dst = dst.astype(np.int64)

    core = dst // Nc
    win = (dst % Nc) // P
    wloc = (dst % Nc) % P
    is_hi = (src >= split).astype(np.int64)

    # bucket[core][win][half] -> list of edge ids
    order = np.lexsort((src, is_hi, win, core))
    so_src, so_dst, so_core, so_win, so_wloc, so_hi = (
        src[order], dst[order], core[order], win[order], wloc[order], is_hi[order])

    counts = np.zeros((n_cores, n_win, 2), dtype=np.int64)
    np.add.at(counts, (so_core, so_win, so_hi), 1)
    cpw = np.ceil(counts / P).astype(np.int64).max(axis=0)  # [n_win, 2]

    # group windows
    groups = []
    slot_global = 0
    lo_col = 0
    hi_col = 0
    for g0 in range(0, n_win, group_windows):
        ws = list(range(g0, min(g0 + group_windows, n_win)))
        g = Plan()
        g.windows = ws
        g.slot0 = slot_global
        g.lo_n = int(sum(cpw[w, 0] for w in ws))
        g.hi_n = int(sum(cpw[w, 1] for w in ws))
        g.n_slots = g.lo_n + g.hi_n
        g.lo_col0 = lo_col          # int16 idx array column offset (cols of 16)
        g.hi_col0 = hi_col
        # per window: local slot indices (within group) for lo and hi chunks
        g.win_slots = {}
        loff, hoff = 0, g.lo_n
        for w in ws:
            sl = list(range(loff, loff + int(cpw[w, 0]))) + \
                 list(range(hoff, hoff + int(cpw[w, 1])))
            g.win_slots[w] = sl
            loff += int(cpw[w, 0])
            hoff += int(cpw[w, 1])
        lo_col += g.lo_n * (P // 16)
        hi_col += g.hi_n * (P // 16)
        slot_global += g.n_slots
        groups.append(g)

    S = slot_global          # total chunk slots per core per layer
    TOT_LO = lo_col * 16
    TOT_HI = hi_col * 16

    # per-core arrays
    idx_lo = np.zeros((n_cores, 16, TOT_LO // 16), dtype=np.int16)
    idx_hi = np.zeros((n_cores, 16, TOT_HI // 16), dtype=np.int16)
    dst16 = np.zeros((n_cores, 16, S * (P // 16)), dtype=np.int16)
    dstloc = np.full((n_cores, P, S), -1.0, dtype=ml_dtypes.bfloat16)

    # edge ranges per (core, win, half) in the sorted order
    start = {}
    pos = 0
    for c in range(n_cores):
        for w in range(n_win):
            for h in range(2):
                cnt = int(counts[c, w, h])
                start[(c, w, h)] = (pos, cnt)
                pos += cnt
    assert pos == len(so_src)

    for c in range(n_cores):
        for g in groups:
            for w in g.windows:
                sl = g.win_slots[w]
                nlo = int(cpw[w, 0])
                for h in (0, 1):
                    base_pos, cnt = start[(c, w, h)]
                    half_slots = sl[:nlo] if h == 0 else sl[nlo:]
                    for j, ls in enumerate(half_slots):
                        s = g.slot0 + ls
                        lo_e = j * P
                        n_e = min(P, cnt - lo_e) if cnt > lo_e else 0
                        if n_e > 0:
                            ee = order_slice = slice(base_pos + lo_e, base_pos + lo_e + n_e)
                            ss = so_src[order_slice]
                            dd = so_dst[order_slice]
                            wl = so_wloc[order_slice]
                        else:
                            ss = dd = wl = np.zeros((0,), np.int64)
                        # fill partitions [0, n_e) with real edges, rest pad
                        pr = np.zeros((P,), np.int64)
                        pr[:n_e] = ss
                        dloc = np.zeros((P,), np.int64)
                        dloc[:n_e] = dd - c * Nc  # local dst < Nc fits int16
                        dst16[c, :, s * (P // 16):(s + 1) * (P // 16)] = \
                            dloc.astype(np.int16).reshape(P // 16, 16).T
                        dl = np.full((P,), -1.0, np.float32)
                        dl[:n_e] = wl
                        dstloc[c, :, s] = dl.astype(ml_dtypes.bfloat16)
                        gidx = pr.copy()
                        if h == 1:
                            gidx = gidx - split
                        gidx[n_e:] = 0
                        # wrap into 16 partitions: element j -> [j%16, j//16]
                        if h == 0:
                            # local col within this gather-call block
                            ls_in_half = ls  # lo slots come first in group
                            col0 = g.lo_col0 + ls_in_half * (P // 16)
                            tgt = idx_lo
                        else:
                            ls_in_half = ls - g.lo_n
                            col0 = g.hi_col0 + ls_in_half * (P // 16)
                            tgt = idx_hi
                        tgt[c, :, col0:col0 + P // 16] = gidx.astype(np.int16).reshape(P // 16, 16).T

    p.N, p.n_cores, p.Nc, p.n_win, p.split = N, n_cores, Nc, n_win, split
    p.groups, p.S, p.TOT_LO, p.TOT_HI = groups, S, TOT_LO, TOT_HI
    p.cpw = cpw
    p.idx_lo = np.tile(idx_lo, (1, 8, 1))   # replicate for 8 Q7 cores -> [n_cores,128,cols]
    p.idx_hi = np.tile(idx_hi, (1, 8, 1))
    p.dst16 = np.tile(dst16, (1, 8, 1))
    p.dstloc = dstloc
    p.win_ndst = [min(P, Nc - w * P) for w in range(n_win)]
    return p


# ----------------------------------------------------------------------------
# Device program emitter
# ----------------------------------------------------------------------------

def emit_gat(tc, outs, ins, plan, macro=4, dbg=False, stop_after=None):
    nc = tc.nc
    N, Nc, n_win, split = plan.N, plan.Nc, plan.n_win, plan.split
    n_cores = plan.n_cores
    S = plan.S
    HC, OUT, H1 = 128, 64, 4
    Smax = max(g.n_slots for g in plan.groups)

    xT = ins["xT"]            # [128, N] bf16
    xT_own = ins["xT_own"]    # [128, Nc] bf16 (per-core dst-shard slice)
    W1aug = ins["W1aug"]      # [128, 192] bf16 = [W1 | a_s mat | a_d mat | 0]
    W2aug = ins["W2aug"]      # [128, 128] bf16 = [W2 | m2s | m2d | 0]
    iota_in = ins["iota"]     # [128, 128] bf16 (row j = 0..127 on free dim)
    ident_in = ins["ident"]   # [128, 128] bf16 identity
    idx_lo_in = ins["idx_lo"]  # [128, TOT_LO//16] i16
    idx_hi_in = ins["idx_hi"]
    dstidx_in = ins["dstidx"]  # [128, S*8] i16 (local dst, gather-wrapped)
    dstloc_in = ins["dstloc"]  # [128, S] bf16
    dstlocF_in = ins["dstlocF"]  # [16, S*128] bf16 (free-major dstloc, 16x rep)
    iotaP_in = ins["iotaP"]    # [128, 512] bf16 (value = partition idx)
    out2 = outs["out2"]       # [Nc, 64] fp32

    ctx = ExitStack()
    with ctx:
        dram = ctx.enter_context(tc.tile_pool(name="dram", bufs=1, space="DRAM"))
        cpool = ctx.enter_context(tc.tile_pool(name="consts", bufs=1))
        bpool = ctx.enter_context(tc.tile_pool(name="build", bufs=3))
        bps = ctx.enter_context(tc.tile_pool(name="bps", bufs=2, space="PSUM"))
        rpool = ctx.enter_context(tc.tile_pool(name="rowsp", bufs=2))
        spool = ctx.enter_context(tc.tile_pool(name="scorep", bufs=2))
        ohpool = ctx.enter_context(tc.tile_pool(name="ohp", bufs=3))
        wps = ctx.enter_context(tc.tile_pool(name="wps", bufs=2, space="PSUM"))
        ops_ = ctx.enter_context(tc.tile_pool(name="ops", bufs=2, space="PSUM"))
        tps = ctx.enter_context(tc.tile_pool(name="tps", bufs=2, space="PSUM"))
        dpool = ctx.enter_context(tc.tile_pool(name="drainp", bufs=2))

        table1 = dram.tile([N, 256], BF16, name="table1")
        table2 = dram.tile([N, 128], BF16, name="table2")
        own_sc1 = dram.tile([Nc, H1], BF16, name="own_sc1")
        own_sc2 = dram.tile([Nc, 1], BF16, name="own_sc2")
        o1T_own = dram.tile([P, Nc], BF16, name="o1T_own")
        o1T_full = dram.tile([P * n_cores, Nc], BF16, name="o1T_full",
                             addr_space="Shared" if n_cores > 4 else "Local")

        # ---- constants to SBUF
        w1_sb = cpool.tile([P, 192], BF16, name="w1_sb")
        nc.sync.dma_start(out=w1_sb[:], in_=W1aug[:])
        w2_sb = cpool.tile([P, 96], BF16, name="w2_sb")
        nc.sync.dma_start(out=w2_sb[:], in_=W2aug[:])
        iota_sb = cpool.tile([P, P], BF16, name="iota_sb")
        nc.sync.dma_start(out=iota_sb[:], in_=iota_in[:])
        ident_sb = cpool.tile([P, P], BF16, name="ident_sb")
        nc.sync.dma_start(out=ident_sb[:], in_=ident_in[:])
        idxlo_sb = cpool.tile([P, plan.TOT_LO // 16], I16, name="idxlo_sb")
        nc.sync.dma_start(out=idxlo_sb[:], in_=idx_lo_in[:])
        idxhi_sb = cpool.tile([P, plan.TOT_HI // 16], I16, name="idxhi_sb")
        nc.sync.dma_start(out=idxhi_sb[:], in_=idx_hi_in[:])
        dsti_sb = cpool.tile([P, S * (P // 16)], I16, name="dsti_sb")
        nc.sync.dma_start(out=dsti_sb[:], in_=dstidx_in[:])
        dstl_sb = cpool.tile([P, S], BF16, name="dstl_sb")
        nc.sync.dma_start(out=dstl_sb[:], in_=dstloc_in[:])
        iotaP_sb = cpool.tile([P, 512], BF16, name="iotaP_sb")
        nc.sync.dma_start(out=iotaP_sb[:], in_=iotaP_in[:])

        # ---- build1: table1 rows [h1 bf16 x128 | fp32 a_s(4) a_d(4) | 0 pad]
        nblk = math.ceil(N / P)
        for b in range(nblk):
            nb = min(P, N - b * P)
            xt = bpool.tile([P, P], BF16, name="xt", tag="xt")
            nc.sync.dma_start(out=xt[:, :nb], in_=xT[:, b * P:b * P + nb])
            ps = bps.tile([P, 192], FP32, name="psb", tag="psb")
            nc.tensor.matmul(out=ps[:nb, :], lhsT=xt[:, :nb], rhs=w1_sb[:],
                             start=True, stop=True)
            t1 = bpool.tile([P, 256], BF16, name="t1", tag="t1")
            nc.vector.tensor_copy(out=t1[:nb, 0:HC], in_=ps[:nb, 0:HC])
            t1f = t1[:].bitcast(FP32)
            nc.vector.tensor_copy(out=t1f[:nb, 64:128], in_=ps[:nb, 128:192])
            nc.sync.dma_start(out=table1[b * P:b * P + nb, :], in_=t1[:nb, :])

        # ---- build own_sc1 rows [a_d1(4) | junk]
        nblk_o = math.ceil(Nc / P)
        for b in range(nblk_o):
            nb = min(P, Nc - b * P)
            xo = bpool.tile([P, P], BF16, name="xo", tag="xt")
            nc.sync.dma_start(out=xo[:, :nb], in_=xT_own[:, b * P:b * P + nb])
            po = bps.tile([P, H1], FP32, name="po", tag="psb")
            nc.tensor.matmul(out=po[:nb, :], lhsT=xo[:, :nb],
                             rhs=w1_sb[:, 132:132 + H1], start=True, stop=True)
            so = bpool.tile([P, H1], BF16, name="so", tag="so")
            nc.vector.tensor_copy(out=so[:nb, :], in_=po[:nb, :])
            nc.sync.dma_start(out=own_sc1[b * P:b * P + nb, :], in_=so[:nb, :])

        if stop_after == "build1":
            nc.gpsimd.dma_start(out=outs["out2"][:, 0:1], in_=table1[0:Nc, 0:2].bitcast(FP32))
            return

        def emit_layer(layer):
            H = H1 if layer == 1 else 1
            F = HC if layer == 1 else OUT
            ROW = 256 if layer == 1 else 128     # table row elems (bf16)
            ASF = 64 if layer == 1 else 32       # fp32 col of embedded a_s
            tab = table1 if layer == 1 else table2
            own = own_sc1 if layer == 1 else own_sc2
            for g in plan.groups:
                Sg = g.n_slots
                rows = rpool.tile([P, Sg, ROW], BF16, name="rows",
                                  tag=f"rows{layer}",
                                  padded_shape=[P, Smax, ROW])
                if "rowg" in DISABLE:
                    nc.vector.memset(rows[:], 0.0)
                if g.lo_n and "rowg" not in DISABLE:
                    nc.gpsimd.dma_gather(
                        out_ap=rows[:, 0:g.lo_n, :],
                        in_ap=tab[0:split, :],
                        idxs_ap=idxlo_sb[:, g.lo_col0:g.lo_col0 + g.lo_n * (P // 16)],
                        num_idxs=g.lo_n * P,
                        num_idxs_reg=g.lo_n * P,
                        elem_size=ROW,
                        single_packet=False,
                    )
                if g.hi_n and "rowg" not in DISABLE:
                    nc.gpsimd.dma_gather(
                        out_ap=rows[:, g.lo_n:g.lo_n + g.hi_n, :],
                        in_ap=tab[split:N, :],
                        idxs_ap=idxhi_sb[:, g.hi_col0:g.hi_col0 + g.hi_n * (P // 16)],
                        num_idxs=g.hi_n * P,
                        num_idxs_reg=g.hi_n * P,
                        elem_size=ROW,
                        single_packet=False,
                    )
                # free-major dstloc broadcast to all partitions (log2 doubling)
                dstF = spool.tile([P, Sg * P], BF16, name="dstF", tag="dstF",
                                  padded_shape=[P, Smax * P])
                nc.sync.dma_start(out=dstF[0:16, :],
                                  in_=dstlocF_in[:, g.slot0 * P:(g.slot0 + Sg) * P])
                r = 16
                while r < P:
                    nc.sync.dma_start(out=dstF[r:2 * r, :], in_=dstF[0:r, :])
                    r *= 2
                # per-edge a_d via ohT matmuls into a group psum
                ado = ops_.tile([P, Sg * H], FP32, name="ado", tag="pso",
                                padded_shape=[P, Smax * H1])
                for w in g.windows:
                    adw = dpool.tile([P, H], BF16, name="adw", tag="adw",
                                     padded_shape=[P, H1])
                    if plan.win_ndst[w] < P:
                        nc.vector.memset(adw[:], 0.0)
                    nc.sync.dma_start(out=adw[:plan.win_ndst[w], :],
                                      in_=own[w * P:w * P + plan.win_ndst[w], :])
                    wslots = g.win_slots[w]
                    runs2 = []
                    for sl in wslots:
                        if runs2 and runs2[-1][-1] == sl - 1:
                            runs2[-1].append(sl)
                        else:
                            runs2.append([sl])
                    for run in runs2:
                        for mi in range(0, len(run), macro):
                            msl = run[mi:mi + macro]
                            C = len(msl)
                            a = msl[0]
                            ohT = ohpool.tile([P, macro, P], BF16, name="ohT",
                                              tag="ohT")
                            nc.vector.tensor_tensor(
                                out=ohT[:, :C, :],
                                in0=iotaP_sb[:, 0:C * P],
                                in1=dstF[:, a * P:(a + C) * P],
                                op=OP.is_equal,
                            )
                            for ci, sl in enumerate(msl):
                                if "admm" in DISABLE:
                                    continue
                                nc.tensor.matmul(out=ado[:, sl * H:(sl + 1) * H],
                                                 lhsT=ohT[:, ci, :], rhs=adw[:],
                                                 start=True, stop=True)
                rows_f = rows[:].bitcast(FP32)   # [P, Sg, ROW//2]
                e_t = spool.tile([P, Sg * H], FP32, name="e_t", tag="e_t",
                                 padded_shape=[P, Smax * H1])
                if "admm" in DISABLE:
                    nc.vector.tensor_copy(out=e_t[:], in_=rows_f[:, :, ASF:ASF + H])
                else:
                    nc.vector.tensor_tensor(out=e_t[:],
                                            in0=rows_f[:, :, ASF:ASF + H],
                                            in1=ado[:], op=OP.add)
                e2_t = spool.tile([P, Sg * H], FP32, name="e2_t", tag="e2_t",
                                  padded_shape=[P, Smax * H1])
                nc.vector.tensor_scalar_mul(out=e2_t[:], in0=e_t[:], scalar1=NEG_SLOPE)
                nc.vector.tensor_tensor(out=e_t[:], in0=e_t[:], in1=e2_t[:], op=OP.max)
                expt = spool.tile([P, Sg, H], BF16, name="expt", tag="expt",
                                  padded_shape=[P, Smax, H1])
                nc.scalar.activation(out=expt[:], in_=e_t[:], func=AF.Exp)

                for w in g.windows:
                    Dw = plan.win_ndst[w]
                    slots = g.win_slots[w]
                    psw = wps.tile([P, F + H], FP32, name="psw", tag="psw",
                                   padded_shape=[P, HC + H1])
                    # split into consecutive runs (lo slots, hi slots), then
                    # into macro-sized batches of consecutive slots
                    runs = []
                    for s in slots:
                        if runs and runs[-1][-1] == s - 1:
                            runs[-1].append(s)
                        else:
                            runs.append([s])
                    macros = []
                    for run in runs:
                        for mi in range(0, len(run), macro):
                            macros.append(run[mi:mi + macro])
                    for mslots in macros:
                        C = len(mslots)
                        a = mslots[0]
                        oh = ohpool.tile([P, macro, P], BF16, name="oh", tag="oh")
                        nc.vector.tensor_tensor(
                            out=oh[:, :C, :],
                            in0=iota_sb[:, None, :].to_broadcast([P, C, P]),
                            in1=dstl_sb[:, g.slot0 + a:g.slot0 + a + C, None].to_broadcast([P, C, P]),
                            op=OP.is_equal,
                        )
                        msg = ohpool.tile([P, macro, F + H], BF16, name="msg", tag="msg",
                                          padded_shape=[P, macro, HC + H1])
                        nc.vector.tensor_tensor(
                            out=msg[:, :C, 0:F],
                            in0=rows[:, a:a + C, 0:F],
                            in1=expt[:, a:a + C, :, None].to_broadcast([P, C, H, F // H]),
                            op=OP.mult,
                        )
                        nc.vector.tensor_copy(out=msg[:, :C, F:F + H],
                                              in_=expt[:, a:a + C, :])
                        for ci, s in enumerate(mslots):
                            first = (s == slots[0])
                            last = (s == slots[-1])
                            nc.tensor.matmul(out=psw[:, :], lhsT=oh[:, ci, :],
                                             rhs=msg[:, ci, :], start=first, stop=last)
                    # drain window
                    den = dpool.tile([P, H], FP32, name="den", tag="den",
                                     padded_shape=[P, H1])
                    nc.vector.tensor_scalar_add(out=den[:], in0=psw[:, F:F + H],
                                                scalar1=1e-16)
                    rec = dpool.tile([P, H], FP32, name="rec", tag="rec",
                                     padded_shape=[P, H1])
                    nc.vector.reciprocal(out=rec[:], in_=den[:])
                    if layer == 1:
                        o1 = dpool.tile([P, HC], FP32, name="o1", tag="o1")
                        nc.vector.tensor_tensor(
                            out=o1[:],
                            in0=psw[:, 0:HC],
                            in1=rec[:, :, None].to_broadcast([P, H, HC // H]),
                            op=OP.mult,
                        )
                        o1b = dpool.tile([P, HC], BF16, name="o1b", tag="o1b")
                        nc.vector.tensor_scalar_max(out=o1b[:], in0=o1[:], scalar1=0.0)
                        pst = tps.tile([P, P], BF16, name="pst", tag="pst")
                        nc.tensor.transpose(out=pst[:], in_=o1b[:], identity=ident_sb[:])
                        o1t = dpool.tile([P, P], BF16, name="o1t", tag="o1t")
                        nc.vector.tensor_copy(out=o1t[:], in_=pst[:])
                        nc.sync.dma_start(out=o1T_own[:, w * P:w * P + Dw],
                                          in_=o1t[:, :Dw])
                        # own a_d2 for layer 2: (relu out1)^T @ m2d
                        if "po2" in DISABLE:
                            continue
                        po2 = ops_.tile([P, 2], FP32, name="po2", tag="pso",
                                        padded_shape=[P, Smax * H1])
                        nc.tensor.matmul(out=po2[:], lhsT=o1t[:],
                                         rhs=w2_sb[:, 64:66], start=True, stop=True)
                        so2 = dpool.tile([P, 1], BF16, name="so2", tag="so2")
                        nc.vector.tensor_copy(out=so2[:], in_=po2[:, 1:2])
                        nc.sync.dma_start(out=own_sc2[w * P:w * P + Dw, :],
                                          in_=so2[:Dw, :])
                    else:
                        o2 = dpool.tile([P, OUT], FP32, name="o2", tag="o2")
                        nc.vector.tensor_scalar(out=o2[:], in0=psw[:, 0:OUT],
                                                scalar1=rec[:, 0:1], scalar2=None,
                                                op0=OP.mult)
                        nc.sync.dma_start(out=out2[w * P:w * P + Dw, :],
                                          in_=o2[:Dw, :])

        emit_layer(1)
        if stop_after == "layer1":
            nc.gpsimd.dma_start(out=outs["out2"][0:P, 0:32], in_=o1T_own[:, 0:64].bitcast(FP32))
            return

        # ---- exchange
        nc.gpsimd.collective_compute(
            "AllGather", OP.bypass,
            replica_groups=[list(range(n_cores))],
            ins=[o1T_own[:]],
            outs=[o1T_full[:]],
        )

        # ---- build2: table2 rows [h2 bf16 x64 | fp32 a_s2 a_d2 | 0 pad]
        nblk2 = math.ceil(Nc / P)
        for r in range(n_cores):
            for b in range(nblk2):
                nb = min(P, Nc - b * P)
                lh = bpool.tile([P, P], BF16, name="xt2", tag="xt")
                nc.sync.dma_start(out=lh[:, :nb],
                                  in_=o1T_full[r * P:(r + 1) * P, b * P:b * P + nb])
                ps = bps.tile([P, 96], FP32, name="psb2", tag="psb")
                nc.tensor.matmul(out=ps[:nb, :], lhsT=lh[:, :nb], rhs=w2_sb[:],
                                 start=True, stop=True)
                t2 = bpool.tile([P, 128], BF16, name="t2", tag="t1")
                nc.vector.tensor_copy(out=t2[:nb, 0:OUT], in_=ps[:nb, 0:OUT])
                t2f = t2[:].bitcast(FP32)
                nc.vector.tensor_copy(out=t2f[:nb, 32:64], in_=ps[:nb, 64:96])
                n0 = r * Nc + b * P
                nc.sync.dma_start(out=table2[n0:n0 + nb, :], in_=t2[:nb, :])

        emit_layer(2)

        if stop_after == "nodbg":
            return
        if dbg:
            nc.gpsimd.dma_start(out=outs["d_table1"][:], in_=table1[:])
            nc.gpsimd.dma_start(out=outs["d_own1"][:], in_=own_sc1[:])
            nc.gpsimd.dma_start(out=outs["d_o1T"][:], in_=o1T_full[:])
            nc.gpsimd.dma_start(out=outs["d_table2"][:], in_=table2[:])
            nc.gpsimd.dma_start(out=outs["d_own2"][:], in_=own_sc2[:])


# ----------------------------------------------------------------------------\n# Host input construction
# ----------------------------------------------------------------------------

def build_host_inputs(plan, x, W1, att_src1, att_dst1, W2, att_src2, att_dst2):
    N = plan.N
    bf = ml_dtypes.bfloat16
    HID = 32
    H1 = att_src1.shape[0]
    m1s = np.stack([W1[:, h * HID:(h + 1) * HID] @ att_src1[h] for h in range(H1)], axis=1)
    m1d = np.stack([W1[:, h * HID:(h + 1) * HID] @ att_dst1[h] for h in range(H1)], axis=1)
    m2s = (W2 @ att_src2[0])[:, None]
    m2d = (W2 @ att_dst2[0])[:, None]
    W2aug = np.zeros((128, 96), np.float32)
    W2aug[:, :64] = W2
    W2aug[:, 64:65] = m2s
    W2aug[:, 65:66] = m2d
    W2aug = W2aug.astype(bf)
    W1p = np.zeros((128, 192), np.float32)
    W1p[:, 0:128] = W1
    W1p[:, 128:132] = m1s
    W1p[:, 132:136] = m1d
    W1aug = W1p.astype(bf)

    xT = np.ascontiguousarray(x.T).astype(bf)  # [128, N]
    iota = np.tile(np.arange(128, dtype=np.float32)[None, :], (128, 1)).astype(bf)
    ident = np.eye(128, dtype=np.float32).astype(bf)

    shared = dict(xT=xT, W1aug=W1aug, W2aug=W2aug, iota=iota, ident=ident)
    in_maps = []
    for c in range(plan.n_cores):
        m = dict(shared)
        m["xT_own"] = np.ascontiguousarray(xT[:, c * plan.Nc:(c + 1) * plan.Nc])
        m["idx_lo"] = plan.idx_lo[c]
        m["idx_hi"] = plan.idx_hi[c]
        m["dstidx"] = plan.dst16[c]
        m["dstloc"] = np.asarray(plan.dstloc[c])
        m["dstlocF"] = np.tile(np.ascontiguousarray(
            np.asarray(plan.dstloc[c]).T).reshape(1, -1), (16, 1))
        m["iotaP"] = np.tile(
            np.arange(128, dtype=np.float32)[:, None], (1, 512)).astype(bf)
        in_maps.append(m)
    return in_maps


def reference_numpy(x, edge_index, W1, att_src1, att_dst1, b1, W2, att_src2,
                    att_dst2, b2):
    N = x.shape[0]

    def lrelu(v):
        return np.where(v > 0, v, NEG_SLOPE * v)

    def gat(xx, src, dst, W, a_s, a_d, b, heads, out_ch, concat):
        n = xx.shape[0]
        h = (xx @ W).reshape(n, heads, out_ch)
        asrc = np.einsum("nhc,hc->nh", h, a_s)
        adst = np.einsum("nhc,hc->nh", h, a_d)
        e = lrelu(asrc[src] + adst[dst])
        m = np.full((n, heads), -np.inf, np.float32)
        np.maximum.at(m, dst, e)
        ex = np.exp(e - m[dst])
        den = np.zeros((n, heads), np.float32)
        np.add.at(den, dst, ex)
        alpha = ex / (den[dst] + 1e-16)
        out = np.zeros((n, heads, out_ch), np.float32)
        np.add.at(out, dst, h[src] * alpha[:, :, None])
        out = out.reshape(n, heads * out_ch) if concat else out.mean(axis=1)
        return out + b

    loop = np.arange(N, dtype=np.int64)
    src = np.concatenate([edge_index[0], loop])
    dst = np.concatenate([edge_index[1], loop])
    h = gat(x, src, dst, W1, att_src1, att_dst1, b1, 4, 32, True)
    h = np.maximum(h, 0)
    return gat(h, src, dst, W2, att_src2, att_dst2, b2, 1, 64, False)


# ----------------------------------------------------------------------------
# Harness entry point
# ----------------------------------------------------------------------------

import os

N_FULL = 50000
N_CORES = 8

LAST_RESULT = None


def _ensure_ntff_hook():
    """Install the axon NTFF profile hook shim if the image lacks
    antenv.axon_hooks (needed only for trace=True)."""
    import sys
    import types
    try:
        import antenv.axon_hooks  # noqa: F401
        return
    except ImportError:
        pass
    mod = types.ModuleType("antenv.axon_hooks")
    state = {}
    mod.set_axon_ntff_profile_hook = lambda h: state.__setitem__("h", h)
    mod.get_axon_ntff_profile_hook = lambda: state.get("h")
    import antenv
    sys.modules["antenv.axon_hooks"] = mod
    antenv.axon_hooks = mod
    try:
        from trn_agent_boot.trn_boot import _ntff_profile_via_ctypes
        hook = _ntff_profile_via_ctypes("/opt/axon/libaxon_pjrt.so")
        if hook is not None:
            mod.set_axon_ntff_profile_hook(hook)
    except Exception as e:  # noqa: BLE001
        print("ntff hook setup failed:", e)


def _build_nc(plan):
    import concourse.bacc as bacc
    nc = bacc.Bacc("TRN2", target_bir_lowering=False, debug=False,
                   num_devices=plan.n_cores)
    ins_t = {
        "xT": nc.dram_tensor("xT", [128, plan.N], BF16, kind="ExternalInput").ap(),
        "W1aug": nc.dram_tensor("W1aug", [128, 192], BF16, kind="ExternalInput").ap(),
        "W2aug": nc.dram_tensor("W2aug", [128, 96], BF16, kind="ExternalInput").ap(),
        "iota": nc.dram_tensor("iota", [128, 128], BF16, kind="ExternalInput").ap(),
        "ident": nc.dram_tensor("ident", [128, 128], BF16, kind="ExternalInput").ap(),
        "idx_lo": nc.dram_tensor("idx_lo", [128, plan.TOT_LO // 16], I16,
                                 kind="ExternalInput").ap(),
        "idx_hi": nc.dram_tensor("idx_hi", [128, plan.TOT_HI // 16], I16,
                                 kind="ExternalInput").ap(),
        "xT_own": nc.dram_tensor("xT_own", [128, plan.Nc], BF16,
                                 kind="ExternalInput").ap(),
        "dstidx": nc.dram_tensor("dstidx", [128, plan.S * 8], I16,
                                 kind="ExternalInput").ap(),
        "dstloc": nc.dram_tensor("dstloc", [128, plan.S], BF16,
                                 kind="ExternalInput").ap(),
        "dstlocF": nc.dram_tensor("dstlocF", [16, plan.S * 128], BF16,
                                  kind="ExternalInput").ap(),
        "iotaP": nc.dram_tensor("iotaP", [128, 512], BF16,
                                kind="ExternalInput").ap(),
    }
    outs_t = {
        "out2": nc.dram_tensor("out2", [plan.Nc, 64], FP32,
                               kind="ExternalOutput").ap(),
    }
    with tile.TileContext(nc) as t:
        emit_gat(t, outs_t, ins_t, plan)
    nc.compile()
    return nc


def kernel(**inputs):
    global LAST_RESULT
    from concourse.bass_utils import run_bass_kernel_spmd

    x = np.asarray(inputs["x"], np.float32)
    edge_index = np.asarray(inputs["edge_index"])
    W1 = np.asarray(inputs["W1"], np.float32)
    as1 = np.asarray(inputs["att_src1"], np.float32)
    ad1 = np.asarray(inputs["att_dst1"], np.float32)
    b1 = np.asarray(inputs["b1"], np.float32)
    W2 = np.asarray(inputs["W2"], np.float32)
    as2 = np.asarray(inputs["att_src2"], np.float32)
    ad2 = np.asarray(inputs["att_dst2"], np.float32)
    b2 = np.asarray(inputs["b2"], np.float32)
    assert float(np.abs(b1).max()) == 0.0, "nonzero b1 not supported"

    N = x.shape[0]
    plan = make_plan(edge_index, N, N_CORES, group_windows=2)
    in_maps = build_host_inputs(plan, x, W1, as1, ad1, W2, as2, ad2)
    nc = _build_nc(plan)
    trace = os.environ.get("GAT_TRACE", "0") == "1"
    if trace:
        _ensure_ntff_hook()
    res = run_bass_kernel_spmd(nc, in_maps, core_ids=list(range(plan.n_cores)),
                               trace=trace)
    LAST_RESULT = res
    out = np.concatenate([res.results[c]["out2"] for c in range(plan.n_cores)],
                         axis=0)
    return (out + b2[None, :]).astype(np.float32)



# revision 12
# speedup vs baseline: 1.5992x; 1.5992x over previous
"""GAT (2-layer, 4-head then 1-head) Bass kernel for TRN2, 8-way graph-parallel.

v2 — gather-paced design. Per core (cores own contiguous dst-node shards):
  - build1 (deduplicated): each core computes table rows only for ITS node
    shard (h1 | a_s | a_d packed in 512B rows), AllGather replicates the full
    table to every core. 8x less build work than replicated builds.
  - aggregation: edges sorted by (group of 2 dst-windows, table-half, window,
    src-row); group-major 128-edge slots (window straddles handled by per-slot
    "piece" lists). dma_gather pulls h[src] rows per slot; self-loop edges are
    NOT gathered (their rows are the core's own contiguous rows -> plain DMA
    into dedicated slots whose one-hot is the identity). Trailing padding uses
    idx=-1 which the gather ucode trims for free.
  - per slot: one-hot matrices from dst-locals via tensor_scalar(is_equal);
    a_d[dst] per edge via ohT^T @ adw matmul; e = Lrelu(a_s+a_d) on ACT;
    exp EXPANDED to all F columns on ACT so the alpha multiply is one clean
    unit-stride DVE op; scatter-add via oh^T @ msg into PSUM per window.
  - layer-1 drain feeds build2 (own shard) directly from SBUF; AllGather
    table2; layer 2 runs the same machinery (H=1, 256B rows).
Output: per-core dst shard [Nc, 64] fp32; host concatenates and adds b2.
"""

import math
import os
from contextlib import ExitStack

import numpy as np
import ml_dtypes

import concourse.bass as bass
import concourse.mybir as mybir
import concourse.tile as tile

P = 128
FP32 = mybir.dt.float32
BF16 = mybir.dt.bfloat16
I16 = mybir.dt.int16
AF = mybir.ActivationFunctionType
OP = mybir.AluOpType

NEG_SLOPE = 0.2
N_FULL = 50000
N_CORES = 8
GW = 2  # windows per group


# ----------------------------------------------------------------------------
# Host-side planning (pure index/structure work; no tensor-value compute)
# ----------------------------------------------------------------------------

class O:
    pass


def make_plan(edge_index: np.ndarray, N: int, n_cores: int):
    p = O()
    assert N % n_cores == 0
    Nc = N // n_cores                    # 6250
    n_win = math.ceil(Nc / P)            # 49
    NcP = n_win * P                      # 6272 (padded shard rows)
    NR = n_cores * NcP                   # 50176 (padded table rows)
    split = NR // 2                      # 25088, multiple of 128
    assert split % P == 0 and split < 32768 and (NR - split) < 32768

    src = edge_index[0].astype(np.int64)
    dst = edge_index[1].astype(np.int64)

    core = dst // Nc
    dl = dst % Nc
    win = dl // P
    grp = win // GW
    gl = dl - grp * GW * P               # group-local dst in [0, GW*128)
    row = (src // Nc) * NcP + (src % Nc)  # padded table row id
    half = (row >= split).astype(np.int64)
    rowh = row - half * split            # idx value (fits int16)

    n_grp = math.ceil(n_win / GW)
    # sort by (core, group, half, window, row)
    order = np.lexsort((row, win, half, grp, core))
    s_core, s_grp, s_half, s_gl, s_rowh = (
        core[order], grp[order], half[order], gl[order], rowh[order])

    counts = np.zeros((n_cores, n_grp, 2), dtype=np.int64)
    np.add.at(counts, (s_core, s_grp, s_half), 1)
    n_slots_gh = np.ceil(counts / P).astype(np.int64).max(axis=0)  # [n_grp, 2]

    # per (core, grp, half) start offsets in sorted arrays
    starts = np.zeros((n_cores, n_grp, 2), dtype=np.int64)
    pos = 0
    for c in range(n_cores):
        for g in range(n_grp):
            for h in range(2):
                starts[c, g, h] = pos
                pos += counts[c, g, h]
    assert pos == len(src)

    # also per (core, grp, half, win) counts for piece computation
    cw = np.zeros((n_cores, n_grp, 2, GW), dtype=np.int64)
    wk = win[order] - grp[order] * GW
    np.add.at(cw, (s_core, s_grp, s_half, wk), 1)

    groups = []
    slot0 = 0
    col0 = 0
    for g in range(n_grp):
        gi = O()
        gi.windows = list(range(g * GW, min((g + 1) * GW, n_win)))
        gi.g = g
        gi.nlo = int(n_slots_gh[g, 0])
        gi.nhi = int(n_slots_gh[g, 1])
        gi.n_self = len(gi.windows)
        gi.n_slots = gi.nlo + gi.nhi + gi.n_self
        gi.slot0 = slot0
        gi.lo_col0 = col0
        gi.hi_col0 = col0 + gi.nlo * (P // 16)
        gi.self_slots = [gi.nlo + gi.nhi + k for k in range(gi.n_self)]
        # memset ranges (slots that may contain junk tails on some core):
        # per half, from min_core(count)//P to n_slots of that half
        gi.memset = []
        for h, (base, nsl) in ((0, (0, gi.nlo)), (1, (gi.nlo, gi.nhi))):
            k0 = int(counts[:, g, h].min()) // P
            if k0 < nsl:
                gi.memset.append((base + k0, base + nsl))
        # pieces per gathered slot: union over cores of windows present
        gi.pieces = []
        for h, (base, nsl) in ((0, (0, gi.nlo)), (1, (gi.nlo, gi.nhi))):
            for i in range(nsl):
                ks = set()
                for c in range(n_cores):
                    # core c's edges in this (g,h) slot i: [i*P, (i+1)*P)
                    # window-k edges occupy [sum(cw[..k']<k), +cw[..k])
                    acc = 0
                    for k in range(len(gi.windows)):
                        a, b = acc, acc + int(cw[c, g, h, k])
                        acc = b
                        if a < (i + 1) * P and b > i * P:
                            ks.add(k)
                gi.pieces.append(sorted(ks))
        for k in range(gi.n_self):
            gi.pieces.append([k])
        col0 += (gi.nlo + gi.nhi) * (P // 16)
        slot0 += gi.n_slots
        groups.append(gi)

    S = slot0
    TOTC = col0  # idx tensor cols (of 16-wrapped)

    idx = np.zeros((n_cores, 16, TOTC), dtype=np.int16)
    dstloc = np.full((n_cores, P, S), -1.0, dtype=np.float32)

    for c in range(n_cores):
        for gi in groups:
            g = gi.g
            for h, (base, nsl, ccol0) in (
                    (0, (0, gi.nlo, gi.lo_col0)), (1, (gi.nlo, gi.nhi, gi.hi_col0))):
                cnt = int(counts[c, g, h])
                b0 = int(starts[c, g, h])
                padv = -1 if os.environ.get("GAT_PADNEG") else 0
                vals = np.full((nsl * P,), padv, dtype=np.int64)
                vals[:cnt] = s_rowh[b0:b0 + cnt]
                idx[c, :, ccol0:ccol0 + nsl * (P // 16)] = (
                    vals.astype(np.int16).reshape(nsl * P // 16, 16).T)
                dv = np.full((nsl * P,), -1.0, dtype=np.float32)
                dv[:cnt] = s_gl[b0:b0 + cnt]
                dstloc[c, :, gi.slot0 + base:gi.slot0 + base + nsl] = (
                    dv.reshape(nsl, P).T)
            for k, w in enumerate(gi.windows):
                nd = min(P, Nc - w * P)
                dv = np.full((P,), -1.0, dtype=np.float32)
                dv[:nd] = k * P + np.arange(nd)
                dstloc[c, :, gi.slot0 + gi.self_slots[k]] = dv

    p.N, p.n_cores, p.Nc, p.n_win, p.NcP, p.NR, p.split = (
        N, n_cores, Nc, n_win, NcP, NR, split)
    p.groups, p.S, p.TOTC, p.n_grp = groups, S, TOTC, n_grp
    p.Smax = max(gi.n_slots for gi in groups)
    p.idx = np.tile(idx, (1, 8, 1))      # replicate for 8 Q7 cores -> [*,128,*]
    p.dstloc = dstloc
    p.win_ndst = [min(P, Nc - w * P) for w in range(n_win)]
    return p


# ----------------------------------------------------------------------------
# Device program emitter
# ----------------------------------------------------------------------------

def emit_gat(tc, outs, ins, plan):
    nc = tc.nc
    DIS = set(filter(None, os.environ.get("GAT_DIS", "").split(",")))
    Nc, n_win, NcP, NR, split = plan.Nc, plan.n_win, plan.NcP, plan.NR, plan.split
    n_cores = plan.n_cores
    Smax = plan.Smax

    xT_own = ins["xT_own"]    # [128, NcP] bf16
    W1aug = ins["W1aug"]      # [128, 192] bf16 = [W1 | m1s | m1d]
    W2aug = ins["W2aug"]      # [128, 96]  bf16 = [W2 | m2s | m2d | pad]
    iota_in = ins["iota"]     # [128, 2*128] bf16 (col j of half k = j + 128k)
    iotaP_in = ins["iotaP"]   # [128, 2*128] bf16 (value = p + 128k)
    ident_in = ins["ident"]   # [128, 128] bf16 identity
    idx_in = ins["idx"]       # [128, TOTC] i16
    dstloc_in = ins["dstloc"]   # [128, S] bf16
    dstlocF_in = ins["dstlocF"]  # [16, S*128] bf16 (free-major, 16x rep)
    out2 = outs["out2"]       # [Nc, 64] fp32

    ctx = ExitStack()
    with ctx:
        dram = ctx.enter_context(tc.tile_pool(name="dram", bufs=1, space="DRAM"))
        cpool = ctx.enter_context(tc.tile_pool(name="consts", bufs=1))
        bpool = ctx.enter_context(tc.tile_pool(name="build", bufs=3))
        bps = ctx.enter_context(tc.tile_pool(name="bps", bufs=2, space="PSUM"))
        rpool = ctx.enter_context(tc.tile_pool(name="rowsp", bufs=3))
        fpool = ctx.enter_context(tc.tile_pool(name="dstfp", bufs=2))
        spool = ctx.enter_context(tc.tile_pool(name="scorep", bufs=2))
        xpool = ctx.enter_context(tc.tile_pool(name="expp", bufs=2))
        ohpool = ctx.enter_context(tc.tile_pool(name="ohp", bufs=3))
        apsum = ctx.enter_context(tc.tile_pool(name="adops", bufs=2, space="PSUM"))
        wps = ctx.enter_context(tc.tile_pool(name="wps", bufs=3, space="PSUM"))
        tps = ctx.enter_context(tc.tile_pool(name="tps", bufs=1, space="PSUM"))
        dpool = ctx.enter_context(tc.tile_pool(name="drainp", bufs=2))

        tab1_own = dram.tile([NcP, 256], BF16, name="tab1_own")
        table1 = dram.tile([NR, 256], BF16, name="table1")
        tab2_own = dram.tile([NcP, 128], BF16, name="tab2_own")
        table2 = dram.tile([NR, 128], BF16, name="table2")

        # ---- constants to SBUF
        w1_sb = cpool.tile([P, 192], BF16, name="w1_sb")
        nc.sync.dma_start(out=w1_sb[:], in_=W1aug[:])
        w2_sb = cpool.tile([P, 96], BF16, name="w2_sb")
        nc.sync.dma_start(out=w2_sb[:], in_=W2aug[:])
        iota_sb = cpool.tile([P, 2, P], BF16, name="iota_sb")
        nc.sync.dma_start(out=iota_sb[:], in_=iota_in[:])
        iotaP_sb = cpool.tile([P, 2, P], BF16, name="iotaP_sb")
        nc.sync.dma_start(out=iotaP_sb[:], in_=iotaP_in[:])
        ident_sb = cpool.tile([P, P], BF16, name="ident_sb")
        nc.sync.dma_start(out=ident_sb[:], in_=ident_in[:])
        idx_sb = cpool.tile([P, plan.TOTC], I16, name="idx_sb")
        nc.sync.dma_start(out=idx_sb[:], in_=idx_in[:])
        dstl_sb = cpool.tile([P, plan.S], FP32, name="dstl_sb")
        nc.sync.dma_start(out=dstl_sb[:], in_=dstloc_in[:])
        adw1_sb = cpool.tile([P, n_win, 4], BF16, name="adw1_sb")
        adw2_sb = cpool.tile([P, n_win, 1], BF16, name="adw2_sb")
        o1T_sb = cpool.tile([P, NcP], BF16, name="o1T_sb")

        # ---- build1 (own shard only): rows [h1 bf16 x128 | a_s,a_d fp32 | pad]
        for b in range(n_win):
            xt = bpool.tile([P, P], BF16, name="xt", tag="xt")
            nc.sync.dma_start(out=xt[:], in_=xT_own[:, b * P:(b + 1) * P])
            ps = bps.tile([P, 192], FP32, name="psb", tag="psb")
            nc.tensor.matmul(out=ps[:], lhsT=xt[:], rhs=w1_sb[:],
                             start=True, stop=True)
            t1 = bpool.tile([P, 256], BF16, name="t1", tag="t1")
            nc.scalar.activation(out=t1[:, 0:128], in_=ps[:, 0:128], func=AF.Copy)
            t1f = t1[:].bitcast(FP32)
            nc.vector.tensor_copy(out=t1f[:, 64:72], in_=ps[:, 128:136])
            nc.vector.tensor_copy(out=adw1_sb[:, b, :], in_=ps[:, 132:136])
            eng = nc.sync if "sdma" in DIS else nc.scalar
            eng.dma_start(out=tab1_own[b * P:(b + 1) * P, :], in_=t1[:])

        stop = os.environ.get("GAT_STOP", "")
        if stop == "build1":
            nc.gpsimd.dma_start(out=out2[:, :],
                                in_=tab1_own[0:Nc, 0:128].bitcast(FP32))
            return

        nc.gpsimd.collective_compute(
            "AllGather", OP.bypass,
            replica_groups=[list(range(n_cores))],
            ins=[tab1_own[:]],
            outs=[table1[:]],
        )
        if stop == "ag1":
            nc.gpsimd.dma_start(out=out2[:, :],
                                in_=table1[0:Nc, 0:128].bitcast(FP32))
            return

        def emit_layer(layer):
            H = 4 if layer == 1 else 1
            F = 128 if layer == 1 else 64
            ROW = 256 if layer == 1 else 128   # table row elems (bf16)
            ASF = 64 if layer == 1 else 32     # fp32 col of embedded a_s
            tab = table1 if layer == 1 else table2
            tab_own = tab1_own if layer == 1 else tab2_own
            adw_sb = adw1_sb if layer == 1 else adw2_sb

            for gi in plan.groups:
                Sg = gi.n_slots
                rows = rpool.tile([P, Sg, ROW], BF16, name="rows",
                                  tag=f"rows{layer}",
                                  padded_shape=[P, Smax, ROW])
                # zero slots that may keep junk tails (before gather overwrite)
                if "memset" in DIS:
                    for (a, b) in gi.memset:
                        nc.vector.memset(rows[:, a:b, :], 0.0)
                # self-loop slots: own contiguous rows, no gather
                if "noself" not in DIS:
                    for k, w in enumerate(gi.windows):
                        nc.sync.dma_start(
                            out=rows[:, gi.self_slots[k], :],
                            in_=tab_own[w * P:(w + 1) * P, :])
                if gi.nlo:
                    nc.gpsimd.dma_gather(
                        out_ap=rows[:, 0:gi.nlo, :],
                        in_ap=tab[0:split, :],
                        idxs_ap=idx_sb[:, gi.lo_col0:gi.lo_col0 + gi.nlo * (P // 16)],
                        num_idxs=gi.nlo * P,
                        num_idxs_reg=gi.nlo * P,
                        elem_size=ROW,
                        single_packet=False,
                    )
                if gi.nhi:
                    nc.gpsimd.dma_gather(
                        out_ap=rows[:, gi.nlo:gi.nlo + gi.nhi, :],
                        in_ap=tab[split:NR, :],
                        idxs_ap=idx_sb[:, gi.hi_col0:gi.hi_col0 + gi.nhi * (P // 16)],
                        num_idxs=gi.nhi * P,
                        num_idxs_reg=gi.nhi * P,
                        elem_size=ROW,
                        single_packet=False,
                    )
                # free-major dst-locals broadcast to 128 partitions
                dstF = fpool.tile([P, Sg * P], BF16, name="dstF", tag="dstF",
                                  padded_shape=[P, Smax * P])
                nc.sync.dma_start(out=dstF[0:16, :],
                                  in_=dstlocF_in[:, gi.slot0 * P:(gi.slot0 + Sg) * P])
                r = 16
                while r < P:
                    nc.sync.dma_start(out=dstF[r:2 * r, :], in_=dstF[0:r, :])
                    r *= 2
                # per-edge a_d via ohT matmuls into a group psum
                ado = apsum.tile([P, Sg, H], FP32, name="ado", tag="ado",
                                 padded_shape=[P, Smax, 4])
                if "ado" not in DIS:
                    for s in range(Sg):
                        ks = gi.pieces[s]
                        for j, k in enumerate(ks):
                            ohT = ohpool.tile([P, P], BF16, name="ohT", tag="ohT")
                            nc.vector.tensor_tensor(
                                out=ohT[:],
                                in0=iotaP_sb[:, k, :],
                                in1=dstF[:, s * P:(s + 1) * P],
                                op=OP.is_equal)
                            nc.tensor.matmul(
                                out=ado[:, s, :], lhsT=ohT[:],
                                rhs=adw_sb[:, gi.windows[k], :],
                                start=(j == 0), stop=(j == len(ks) - 1))
                # e = a_s[src] + a_d[dst]; Lrelu on ACT; exp expanded to F
                rows_f = rows[:].bitcast(FP32)     # [P, Sg, ROW//2]
                e_t = spool.tile([P, Sg, H], FP32, name="e_t", tag="e_t",
                                 padded_shape=[P, Smax, 4])
                if "ado" in DIS:
                    nc.vector.tensor_copy(out=e_t[:], in_=rows_f[:, :, ASF:ASF + H])
                else:
                    nc.vector.tensor_tensor(out=e_t[:], in0=rows_f[:, :, ASF:ASF + H],
                                            in1=ado[:], op=OP.add)
                l_t = spool.tile([P, Sg, H], FP32, name="l_t", tag="l_t",
                                 padded_shape=[P, Smax, 4])
                nc.vector.tensor_scalar_mul(out=l_t[:], in0=e_t[:],
                                            scalar1=NEG_SLOPE)
                nc.vector.tensor_tensor(out=l_t[:], in0=e_t[:], in1=l_t[:],
                                        op=OP.max)
                expF = xpool.tile([P, Sg, H, F // H], BF16, name="expF",
                                  tag=f"expF{layer}",
                                  padded_shape=[P, Smax, H, F // H])
                if "expact" in DIS:
                    expS = xpool.tile([P, Sg, H], BF16, name="expS", tag="expS",
                                      padded_shape=[P, Smax, 4])
                    nc.scalar.activation(out=expS[:], in_=l_t[:], func=AF.Exp)
                else:
                    nc.scalar.activation(
                        out=expF[:],
                        in_=l_t[:, :, :, None].to_broadcast([P, Sg, H, F // H]),
                        func=AF.Exp)
                # per-window scatter state
                psw = {}
                first = {}
                npc = [0] * len(gi.windows)
                for s in range(Sg):
                    for k in gi.pieces[s]:
                        npc[k] += 1
                for k, w in enumerate(gi.windows):
                    psw[k] = wps.tile([P, F + H], FP32, name="psw", tag="psw",
                                      padded_shape=[P, 132])
                    first[k] = True
                done = [0] * len(gi.windows)
                for s in range(Sg if "scat" not in DIS else 0):
                    msg = ohpool.tile([P, F + H], BF16, name="msg", tag="msg",
                                      padded_shape=[P, 132])
                    if "expact" in DIS:
                        nc.vector.tensor_tensor(
                            out=msg[:, 0:F], in0=rows[:, s, 0:F],
                            in1=expS[:, s, :, None].to_broadcast([P, H, F // H]),
                            op=OP.mult)
                        nc.vector.tensor_copy(out=msg[:, F:F + H], in_=expS[:, s, :])
                    else:
                        nc.vector.tensor_tensor(out=msg[:, 0:F], in0=rows[:, s, 0:F],
                                                in1=expF[:, s, :, :], op=OP.mult)
                        nc.vector.tensor_copy(out=msg[:, F:F + H], in_=expF[:, s, :, 0])
                    for k in gi.pieces[s]:
                        oh = ohpool.tile([P, P], BF16, name="oh", tag="oh")
                        nc.vector.tensor_scalar(
                            out=oh[:], in0=iota_sb[:, k, :],
                            scalar1=dstl_sb[:, gi.slot0 + s:gi.slot0 + s + 1],
                            scalar2=None, op0=OP.is_equal)
                        done[k] += 1
                        nc.tensor.matmul(out=psw[k][:], lhsT=oh[:], rhs=msg[:],
                                         start=first[k], stop=(done[k] == npc[k]))
                        first[k] = False
                # drain windows
                for k, w in enumerate(gi.windows if "drain" not in DIS else []):
                    Dw = plan.win_ndst[w]
                    den = dpool.tile([P, H], FP32, name="den", tag="den",
                                     padded_shape=[P, 4])
                    nc.vector.tensor_scalar_add(out=den[:], in0=psw[k][:, F:F + H],
                                                scalar1=1e-16)
                    rec = dpool.tile([P, H], FP32, name="rec", tag="rec",
                                     padded_shape=[P, 4])
                    nc.vector.reciprocal(out=rec[:], in_=den[:])
                    if layer == 1:
                        o1 = dpool.tile([P, 128], FP32, name="o1", tag="o1")
                        for h in range(H):
                            nc.vector.tensor_scalar(
                                out=o1[:, h * 32:(h + 1) * 32],
                                in0=psw[k][:, h * 32:(h + 1) * 32],
                                scalar1=rec[:, h:h + 1], scalar2=None, op0=OP.mult)
                        o1b = dpool.tile([P, 128], BF16, name="o1b", tag="o1b")
                        nc.vector.tensor_scalar_max(out=o1b[:], in0=o1[:], scalar1=0.0)
                        pst = tps.tile([P, P], BF16, name="pst", tag="pst")
                        nc.tensor.transpose(out=pst[:], in_=o1b[:], identity=ident_sb[:])
                        if "actcopy" in DIS:
                            nc.vector.tensor_copy(out=o1T_sb[:, w * P:(w + 1) * P],
                                                  in_=pst[:])
                        else:
                            nc.scalar.activation(out=o1T_sb[:, w * P:(w + 1) * P],
                                                 in_=pst[:], func=AF.Copy)
                        # build2 for this window, straight from SBUF
                        ps2 = bps.tile([P, 96], FP32, name="ps2", tag="psb")
                        nc.tensor.matmul(out=ps2[:], lhsT=o1T_sb[:, w * P:(w + 1) * P],
                                         rhs=w2_sb[:], start=True, stop=True)
                        t2 = bpool.tile([P, 128], BF16, name="t2", tag="t1")
                        nc.scalar.activation(out=t2[:, 0:64], in_=ps2[:, 0:64],
                                             func=AF.Copy)
                        t2f = t2[:].bitcast(FP32)
                        nc.vector.tensor_copy(out=t2f[:, 32:34], in_=ps2[:, 64:66])
                        nc.vector.tensor_copy(out=adw2_sb[:, w, :], in_=ps2[:, 65:66])
                        eng2 = nc.sync if "sdma" in DIS else nc.scalar
                        eng2.dma_start(out=tab2_own[w * P:(w + 1) * P, :], in_=t2[:])
                    else:
                        o2 = dpool.tile([P, 64], FP32, name="o2", tag="o2")
                        nc.vector.tensor_scalar(out=o2[:], in0=psw[k][:, 0:64],
                                                scalar1=rec[:, 0:1], scalar2=None,
                                                op0=OP.mult)
                        nc.sync.dma_start(out=out2[w * P:w * P + Dw, :],
                                          in_=o2[:Dw, :])

        emit_layer(1)
        if stop == "l1":
            nc.gpsimd.dma_start(out=out2[:, :],
                                in_=tab2_own[0:Nc, 0:128].bitcast(FP32))
            return

        nc.gpsimd.collective_compute(
            "AllGather", OP.bypass,
            replica_groups=[list(range(n_cores))],
            ins=[tab2_own[:]],
            outs=[table2[:]],
        )
        if stop == "ag2":
            nc.gpsimd.dma_start(out=out2[:, :],
                                in_=table2[0:Nc, 0:128].bitcast(FP32))
            return

        emit_layer(2)


# ----------------------------------------------------------------------------
# Host input construction
# ----------------------------------------------------------------------------

def build_host_inputs(plan, x, W1, att_src1, att_dst1, W2, att_src2, att_dst2):
    bf = ml_dtypes.bfloat16
    HID = 32
    H1 = att_src1.shape[0]
    m1s = np.stack([W1[:, h * HID:(h + 1) * HID] @ att_src1[h] for h in range(H1)], axis=1)
    m1d = np.stack([W1[:, h * HID:(h + 1) * HID] @ att_dst1[h] for h in range(H1)], axis=1)
    m2s = (W2 @ att_src2[0])[:, None]
    m2d = (W2 @ att_dst2[0])[:, None]
    W1aug = np.zeros((128, 192), np.float32)
    W1aug[:, 0:128] = W1
    W1aug[:, 128:132] = m1s
    W1aug[:, 132:136] = m1d
    W1aug = W1aug.astype(bf)
    W2aug = np.zeros((128, 96), np.float32)
    W2aug[:, :64] = W2
    W2aug[:, 64:65] = m2s
    W2aug[:, 65:66] = m2d
    W2aug = W2aug.astype(bf)

    xT = np.ascontiguousarray(x.T).astype(bf)  # [128, N]
    iota = np.zeros((128, 2, 128), np.float32)
    iota[:, 0, :] = np.arange(128)[None, :]
    iota[:, 1, :] = 128 + np.arange(128)[None, :]
    iota = iota.reshape(128, 256).astype(bf)
    iotaP = np.zeros((128, 2, 128), np.float32)
    iotaP[:, 0, :] = np.arange(128)[:, None]
    iotaP[:, 1, :] = 128 + np.arange(128)[:, None]
    iotaP = iotaP.reshape(128, 256).astype(bf)
    ident = np.eye(128, dtype=np.float32).astype(bf)

    shared = dict(W1aug=W1aug, W2aug=W2aug, iota=iota, iotaP=iotaP, ident=ident)
    in_maps = []
    for c in range(plan.n_cores):
        m = dict(shared)
        xo = np.zeros((128, plan.NcP), np.float32)
        xo[:, :plan.Nc] = xT[:, c * plan.Nc:(c + 1) * plan.Nc].astype(np.float32)
        m["xT_own"] = xo.astype(bf)
        m["idx"] = plan.idx[c]
        m["dstloc"] = np.asarray(plan.dstloc[c], np.float32)
        m["dstlocF"] = np.tile(np.ascontiguousarray(
            plan.dstloc[c].astype(ml_dtypes.bfloat16).T).reshape(1, -1), (16, 1))
        in_maps.append(m)
    return in_maps


# ----------------------------------------------------------------------------
# Harness entry point
# ----------------------------------------------------------------------------

LAST_RESULT = None


def _ensure_ntff_hook():
    import sys
    import types
    try:
        import antenv.axon_hooks  # noqa: F401
        return
    except ImportError:
        pass
    mod = types.ModuleType("antenv.axon_hooks")
    state = {}
    mod.set_axon_ntff_profile_hook = lambda h: state.__setitem__("h", h)
    mod.get_axon_ntff_profile_hook = lambda: state.get("h")
    import antenv
    sys.modules["antenv.axon_hooks"] = mod
    antenv.axon_hooks = mod
    try:
        from trn_agent_boot.trn_boot import _ntff_profile_via_ctypes
        hook = _ntff_profile_via_ctypes("/opt/axon/libaxon_pjrt.so")
        if hook is not None:
            mod.set_axon_ntff_profile_hook(hook)
    except Exception as e:  # noqa: BLE001
        print("ntff hook setup failed:", e)


def _build_nc(plan):
    import concourse.bacc as bacc
    nc = bacc.Bacc("TRN2", target_bir_lowering=False, debug=False,
                   num_devices=plan.n_cores)
    ins_t = {
        "xT_own": nc.dram_tensor("xT_own", [128, plan.NcP], BF16,
                                 kind="ExternalInput").ap(),
        "W1aug": nc.dram_tensor("W1aug", [128, 192], BF16, kind="ExternalInput").ap(),
        "W2aug": nc.dram_tensor("W2aug", [128, 96], BF16, kind="ExternalInput").ap(),
        "iota": nc.dram_tensor("iota", [128, 256], BF16, kind="ExternalInput").ap(),
        "iotaP": nc.dram_tensor("iotaP", [128, 256], BF16, kind="ExternalInput").ap(),
        "ident": nc.dram_tensor("ident", [128, 128], BF16, kind="ExternalInput").ap(),
        "idx": nc.dram_tensor("idx", [128, plan.TOTC], I16,
                              kind="ExternalInput").ap(),
        "dstloc": nc.dram_tensor("dstloc", [128, plan.S], FP32,
                                 kind="ExternalInput").ap(),
        "dstlocF": nc.dram_tensor("dstlocF", [16, plan.S * 128], BF16,
                                  kind="ExternalInput").ap(),
    }
    outs_t = {
        "out2": nc.dram_tensor("out2", [plan.Nc, 64], FP32,
                               kind="ExternalOutput").ap(),
    }
    with tile.TileContext(nc) as t:
        emit_gat(t, outs_t, ins_t, plan)
    nc.compile()
    return nc


def kernel(**inputs):
    global LAST_RESULT
    from concourse.bass_utils import run_bass_kernel_spmd

    x = np.asarray(inputs["x"], np.float32)
    edge_index = np.asarray(inputs["edge_index"])
    W1 = np.asarray(inputs["W1"], np.float32)
    as1 = np.asarray(inputs["att_src1"], np.float32)
    ad1 = np.asarray(inputs["att_dst1"], np.float32)
    b1 = np.asarray(inputs["b1"], np.float32)
    W2 = np.asarray(inputs["W2"], np.float32)
    as2 = np.asarray(inputs["att_src2"], np.float32)
    ad2 = np.asarray(inputs["att_dst2"], np.float32)
    b2 = np.asarray(inputs["b2"], np.float32)
    assert float(np.abs(b1).max()) == 0.0, "nonzero b1 not supported"

    N = x.shape[0]
    plan = make_plan(edge_index, N, N_CORES)
    in_maps = build_host_inputs(plan, x, W1, as1, ad1, W2, as2, ad2)
    nc = _build_nc(plan)
    trace = os.environ.get("GAT_TRACE", "0") == "1"
    if trace:
        _ensure_ntff_hook()
    res = run_bass_kernel_spmd(nc, in_maps, core_ids=list(range(plan.n_cores)),
                               trace=trace)
    LAST_RESULT = res
    out = np.concatenate([res.results[c]["out2"] for c in range(plan.n_cores)],
                         axis=0)
    return (out + b2[None, :]).astype(np.float32)
